# revision 19
# baseline (speedup 1.0000x reference)
"""MoE top-2 routing kernel for trn2, expert-parallel over 8 cores.

B=2, S=1024, D=1024, H=4096, E=8, K=2.  Each core hosts one expert
(pre-transposed bf16 weights), computes the f32 router replicated,
compacts its routed tokens (<=C=640) via cumsum + indirect-DMA scatter,
gathers token rows, runs fc1/relu/fc2 in bf16 (prob-scaling folded into
the gathered activations), scatters partial outputs back to token rows.
Host sums the 8 partials. aux_loss computed on device (core 0's copy).
"""

import numpy as np
import ml_dtypes

import concourse.bass as bass
import concourse.bacc as bacc
import concourse.tile as tile
import concourse.mybir as mybir
from concourse import bass_utils
from concourse.bass import IndirectOffsetOnAxis
from concourse.masks import make_identity

F32 = mybir.dt.float32
BF16 = mybir.dt.bfloat16
I32 = mybir.dt.int32
U32 = mybir.dt.uint32
AF = mybir.ActivationFunctionType
ALU = mybir.AluOpType


def mm_split(nc, out, lhsT, rhs, start, stop, nmax=512):
    """matmul with out columns split so each inst stays within a PSUM bank."""
    n = out.shape[-1]
    for s in range(0, n, nmax):
        e = min(s + nmax, n)
        nc.tensor.matmul(out[:, s:e], lhsT=lhsT, rhs=rhs[:, s:e],
                         start=start, stop=stop)


class Cfg:
    def __init__(self, N=2048, D=1024, H=4096, E=8, C=640):
        self.N, self.D, self.H, self.E, self.C = N, D, H, E, C
        self.NCH = N // 128      # token chunks
        self.ND = D // 128       # d chunks
        self.NH = H // 128       # h chunks
        self.NS = C // 128       # slot tiles
        self.NPAD = N + 128      # x rows incl. dump row at N
        self.DUMP = N            # dump row index


def build_moe(tc: tile.TileContext, cfg: Cfg, io: dict, dbg: dict | None = None):
    nc = tc.nc
    N, D, H, E, C = cfg.N, cfg.D, cfg.H, cfg.E, cfg.C
    NCH, ND, NH, NS = cfg.NCH, cfg.ND, cfg.NH, cfg.NS

    xpad, xT, wgT = io["xpad"], io["xT"], io["wgT"]
    w1t, w2t, b1c, b2r = io["w1t"], io["w2t"], io["b1c"], io["b2r"]
    e0f, io8, iotok = io["e0f"], io["io8"], io["iotok"]
    part, aux = io["part"], io["aux"]

    with tc.tile_pool(name="persist", bufs=1) as pp, \
         tc.tile_pool(name="w1pool", bufs=1) as pw1, \
         tc.tile_pool(name="dram", bufs=1, space="DRAM") as pdram:
        # ---- constants / small inputs ----
        ident_f = pp.tile([128, 128], F32, tag="identf")
        make_identity(nc, ident_f[:])
        ident_b = pp.tile([128, 128], BF16, tag="identb")
        make_identity(nc, ident_b[:])
        # strict upper-tri ones [NCH, NCH]: L[i,j]=1 iff i<j (for excl cumsum)
        lst = pp.tile([NCH, NCH], F32, tag="lst")
        nc.sync.dma_start(lst[:], io["ltri"][:])
        zrow = pp.tile([NCH, 128], F32, tag="zrow")
        nc.vector.memset(zrow[:], 0.0)
        ones = pp.tile([128, 1], F32, tag="ones")
        nc.vector.memset(ones[:], 1.0)

        b1sb = pp.tile([128, NH], F32, tag="b1sb")
        nc.sync.dma_start(b1sb[:].rearrange("p (c o) -> p c o", o=1),
                          b1c.rearrange("(c p) o -> p c o", p=128))
        b2sb = pp.tile([1, D], F32, tag="b2sb")
        nc.sync.dma_start(b2sb[:], b2r[:])
        e0sb = pp.tile([128, 1], F32, tag="e0sb")
        nc.sync.dma_start(e0sb[:], e0f[:])
        io8sb = pp.tile([128, E], F32, tag="io8sb")
        nc.sync.dma_start(io8sb[:], io8[:])
        itok = pp.tile([128, NCH], F32, tag="itok")
        nc.sync.dma_start(itok[:], iotok[:])
        wgsb = pp.tile([128, ND * E], F32, tag="wgsb")
        nc.sync.dma_start(wgsb[:].rearrange("p (c e) -> p c e", e=cfg.E),
                          wgT.rearrange("(c p) e -> p c e", p=128))

        # w1 resident: ND tiles [128, H] bf16
        w1sb = []
        for d in range(ND):
            t = pw1.tile([128, H], BF16, tag=f"w1_{d}")
            nc.sync.dma_start(t[:], w1t[d * 128:(d + 1) * 128, :])
            w1sb.append(t)

        # =================== phase R: router + tables ===================
        with tc.tile_pool(name="pr_logit", bufs=1, space="PSUM") as prl, \
             tc.tile_pool(name="pr_small", bufs=2, space="PSUM") as prs, \
             tc.tile_pool(name="sb_xt", bufs=2) as sxt, \
             tc.tile_pool(name="sb_lg", bufs=1) as slg:
            # logits [E, N] = wgT.T @ xT  (f32)
            lps = prl.tile([E, N], F32, tag="lps")
            for d in range(ND):
                xts = sxt.tile([128, N], F32, tag="xt")
                nc.sync.dma_start(xts[:], xT[d * 128:(d + 1) * 128, :])
                for t4 in range(N // 512):
                    nc.tensor.matmul(
                        lps[:, t4 * 512:(t4 + 1) * 512],
                        lhsT=wgsb[:, d * E:(d + 1) * E],
                        rhs=xts[:, t4 * 512:(t4 + 1) * 512],
                        start=(d == 0), stop=(d == ND - 1))
            lcopy = slg.tile([E, N], F32, tag="lcopy")
            nc.vector.tensor_copy(lcopy[:], lps[:])
            # transpose to [t, e] per 128-token chunk
            lte = pp.tile([128, NCH * E], F32, tag="lte")
            for c in range(NCH):
                pt = prs.tile([128, E], F32, tag="tp")
                nc.tensor.transpose(pt[:], lcopy[:, c * 128:(c + 1) * 128],
                                    ident_f[0:E, 0:E])
                nc.vector.tensor_copy(lte[:, c * E:(c + 1) * E], pt[:])

            # top-2 (sorted top-8) per token
            mv = pp.tile([128, NCH * E], F32, tag="mv")
            mi = pp.tile([128, NCH * E], U32, tag="mi")
            for c in range(NCH):
                sl = slice(c * E, (c + 1) * E)
                nc.vector.max(out=mv[:, sl], in_=lte[:, sl])
                nc.vector.max_index(out=mi[:, sl], in_max=mv[:, sl],
                                    in_values=lte[:, sl])
            mv3 = mv[:].rearrange("p (c e) -> p c e", e=E)
            mi3 = mi[:].rearrange("p (c e) -> p c e", e=E)
            e1f = pp.tile([128, NCH], F32, tag="e1f")
            e2f = pp.tile([128, NCH], F32, tag="e2f")
            nc.vector.tensor_copy(e1f[:], mi3[:, :, 0])
            nc.vector.tensor_copy(e2f[:], mi3[:, :, 1])
            d21 = pp.tile([128, NCH], F32, tag="d21")
            nc.vector.tensor_tensor(out=d21[:], in0=mv3[:, :, 1],
                                    in1=mv3[:, :, 0], op=ALU.subtract)
            p2 = pp.tile([128, NCH], F32, tag="p2")
            nc.scalar.activation(p2[:], d21[:], AF.Sigmoid)
            p1 = pp.tile([128, NCH], F32, tag="p1")
            nc.scalar.activation(p1[:], p2[:], AF.Copy, bias=1.0, scale=-1.0)

            # my-expert mask + prob
            my1 = pp.tile([128, NCH], F32, tag="my1")
            my2 = pp.tile([128, NCH], F32, tag="my2")
            nc.vector.tensor_scalar(out=my1[:], in0=e1f[:], scalar1=e0sb[:, 0:1],
                                    scalar2=None, op0=ALU.is_equal)
            nc.vector.tensor_scalar(out=my2[:], in0=e2f[:], scalar1=e0sb[:, 0:1],
                                    scalar2=None, op0=ALU.is_equal)
            mym = pp.tile([128, NCH], F32, tag="mym")
            nc.vector.tensor_add(mym[:], my1[:], my2[:])
            t1 = pp.tile([128, NCH], F32, tag="t1")
            t2 = pp.tile([128, NCH], F32, tag="t2")
            nc.vector.tensor_mul(t1[:], my1[:], p1[:])
            nc.vector.tensor_mul(t2[:], my2[:], p2[:])
            myrp = pp.tile([128, NCH], F32, tag="myrp")
            nc.vector.tensor_add(myrp[:], t1[:], t2[:])

            # ---- aux loss ----
            sh = pp.tile([128, NCH * E], F32, tag="sh")
            for c in range(NCH):
                sl = slice(c * E, (c + 1) * E)
                nc.vector.tensor_scalar(out=sh[:, sl], in0=lte[:, sl],
                                        scalar1=mv[:, c * E:c * E + 1],
                                        scalar2=None, op0=ALU.subtract)
            ep = pp.tile([128, NCH * E], F32, tag="ep")
            nc.scalar.activation(ep[:], sh[:], AF.Exp)
            zs = pp.tile([128, NCH], F32, tag="zs")
            nc.vector.reduce_sum(
                out=zs[:].rearrange("p (c o) -> p c o", o=1),
                in_=ep[:].rearrange("p (c e) -> p c e", e=E),
                axis=mybir.AxisListType.X)
            zi = pp.tile([128, NCH], F32, tag="zi")
            nc.vector.reciprocal(zi[:], zs[:])
            prb = pp.tile([128, NCH * E], F32, tag="prb")
            for c in range(NCH):
                sl = slice(c * E, (c + 1) * E)
                nc.vector.tensor_scalar(out=prb[:, sl], in0=ep[:, sl],
                                        scalar1=zi[:, c:c + 1],
                                        scalar2=None, op0=ALU.mult)
            pracc = pp.tile([128, E], F32, tag="pracc")
            nc.vector.reduce_sum(
                out=pracc[:].rearrange("p (e o) -> p e o", o=1),
                in_=prb[:].rearrange("p (c e) -> p e c", e=E),
                axis=mybir.AxisListType.X)
            eq1 = pp.tile([128, NCH * E], F32, tag="eq1")
            eq2 = pp.tile([128, NCH * E], F32, tag="eq2")
            for c in range(NCH):
                sl = slice(c * E, (c + 1) * E)
                nc.vector.tensor_scalar(out=eq1[:, sl], in0=io8sb[:],
                                        scalar1=e1f[:, c:c + 1],
                                        scalar2=None, op0=ALU.is_equal)
                nc.vector.tensor_scalar(out=eq2[:, sl], in0=io8sb[:],
                                        scalar1=e2f[:, c:c + 1],
                                        scalar2=None, op0=ALU.is_equal)
            eqs = pp.tile([128, NCH * E], F32, tag="eqs")
            nc.vector.tensor_add(eqs[:], eq1[:], eq2[:])
            cnt8 = pp.tile([128, E], F32, tag="cnt8")
            nc.vector.reduce_sum(
                out=cnt8[:].rearrange("p (e o) -> p e o", o=1),
                in_=eqs[:].rearrange("p (c e) -> p e c", e=E),
                axis=mybir.AxisListType.X)
            psa = prs.tile([1, 2 * E], F32, tag="tp")
            nc.tensor.matmul(psa[:, 0:E], lhsT=ones[:], rhs=pracc[:],
                             start=True, stop=True)
            nc.tensor.matmul(psa[:, E:2 * E], lhsT=ones[:], rhs=cnt8[:],
                             start=True, stop=True)
            acop = pp.tile([1, 2 * E], F32, tag="acop")
            nc.vector.tensor_copy(acop[:], psa[:])
            am = pp.tile([1, E], F32, tag="am")
            nc.vector.tensor_mul(am[:], acop[:, 0:E], acop[:, E:2 * E])
            asm = pp.tile([1, 1], F32, tag="asm")
            nc.vector.reduce_sum(out=asm[:], in_=am[:],
                                 axis=mybir.AxisListType.X)
            auxsb = pp.tile([1, 1], F32, tag="auxsb")
            nc.scalar.activation(auxsb[:], asm[:], AF.Copy,
                                 scale=float(E) / float(N) / float(N))
            nc.sync.dma_start(aux[:], auxsb[:])

            # ---- compaction: global positions of my tokens ----
            pmt = prs.tile([NCH, 128], F32, tag="tpw")
            nc.tensor.transpose(pmt[:], mym[:], ident_f[:])
            mmt = pp.tile([NCH, 128], F32, tag="mmt")
            nc.vector.tensor_copy(mmt[:], pmt[:])
            posw = pp.tile([NCH, 128], F32, tag="posw")
            nc.vector.tensor_tensor_scan(
                out=posw[:], data0=mmt[:], data1=zrow[:], initial=0.0,
                op0=ALU.add, op1=ALU.max)
            tot = pp.tile([NCH, 1], F32, tag="tot")
            nc.vector.tensor_copy(tot[:], posw[:, 127:128])
            pb = prs.tile([NCH, 1], F32, tag="tpw")
            nc.tensor.matmul(pb[:], lhsT=lst[:], rhs=tot[:], start=True,
                             stop=True)
            base = pp.tile([NCH, 1], F32, tag="base")
            nc.vector.tensor_copy(base[:], pb[:])
            posg = pp.tile([NCH, 128], F32, tag="posg")
            nc.vector.tensor_scalar(out=posg[:], in0=posw[:],
                                    scalar1=base[:, 0:1], scalar2=None,
                                    op0=ALU.add)
            pposf = prs.tile([128, NCH], F32, tag="tp")
            nc.tensor.transpose(pposf[:], posg[:], ident_f[0:NCH, 0:NCH])
            posf = pp.tile([128, NCH], F32, tag="posf")
            nc.vector.tensor_copy(posf[:], pposf[:])

            # offsets: mym ? pos-1 : DUMP
            o1 = pp.tile([128, NCH], F32, tag="o1")
            nc.vector.tensor_scalar(out=o1[:], in0=posf[:],
                                    scalar1=-(1.0 + cfg.DUMP), scalar2=None,
                                    op0=ALU.add)
            o2 = pp.tile([128, NCH], F32, tag="o2")
            nc.vector.tensor_mul(o2[:], o1[:], mym[:])
            off = pp.tile([128, NCH], F32, tag="off")
            nc.vector.tensor_scalar(out=off[:], in0=o2[:],
                                    scalar1=float(cfg.DUMP), scalar2=None,
                                    op0=ALU.add)
            offi = pp.tile([128, NCH], I32, tag="offi")
            nc.vector.tensor_copy(offi[:], off[:])

            # (tokenid, prob) pairs
            vals = pp.tile([128, NCH * 2], F32, tag="vals")
            v3 = vals[:].rearrange("p (c v) -> p c v", v=2)
            nc.vector.tensor_copy(v3[:, :, 0], itok[:])
            nc.vector.tensor_copy(v3[:, :, 1], myrp[:])

            # table init: every row (DUMP, 0.0)
            initp = pp.tile([128, 34], F32, tag="initp")
            i3 = initp[:].rearrange("p (r v) -> p r v", v=2)
            nc.vector.memset(i3[:, :, 0:1], float(cfg.DUMP))
            nc.vector.memset(i3[:, :, 1:2], 0.0)
            nrep = cfg.NPAD // 128
            table = pdram.tile([cfg.NPAD, 2], F32, tag="table")
            nc.gpsimd.dma_start(
                table[:].rearrange("(p r) v -> p r v", p=128),
                initp[:].rearrange("p (r v) -> p r v", v=2)[:, 0:nrep, :])
            v3w = vals[:].rearrange("p (c v) -> p c v", v=2)
            for c in range(NCH):
                nc.gpsimd.indirect_dma_start(
                    out=table[:],
                    out_offset=IndirectOffsetOnAxis(ap=offi[:, c:c + 1],
                                                    axis=0),
                    in_=v3w[:, c, :],
                    in_offset=None)

            # read back slot tables
            pairs, idxs = [], []
            for k in range(NS):
                pk = pp.tile([128, 2], F32, tag=f"pair{k}")
                nc.sync.dma_start(pk[:], table[k * 128:(k + 1) * 128, :])
                ik = pp.tile([128, 1], I32, tag=f"idx{k}")
                nc.vector.tensor_copy(ik[:], pk[:, 0:1])
                pairs.append(pk)
                idxs.append(ik)

            if dbg is not None:
                nc.sync.dma_start(dbg["d_p1"][:], p1[:])
                nc.sync.dma_start(dbg["d_p2"][:], p2[:])
                nc.sync.dma_start(dbg["d_myrp"][:], myrp[:])
                nc.sync.dma_start(dbg["d_mym"][:], mym[:])
                nc.sync.dma_start(dbg["d_posf"][:], posf[:])
                nc.sync.dma_start(dbg["d_offi"][:], offi[:])
                nc.sync.dma_start(dbg["d_vals"][:], vals[:])
                for k in range(NS):
                    nc.sync.dma_start(
                        dbg["d_tab"][k * 128:(k + 1) * 128, :], pairs[k][:])

        # =================== phase F: gather, fc1, fc2, scatter =========
        with tc.tile_pool(name="pf_mm", bufs=3, space="PSUM") as pfm, \
             tc.tile_pool(name="pf_small", bufs=2, space="PSUM") as pfs, \
             tc.tile_pool(name="sb_xg", bufs=2) as sxg, \
             tc.tile_pool(name="sb_xgb", bufs=2) as sxb, \
             tc.tile_pool(name="sb_w2", bufs=10) as sw2:
            # gather + prob-scale + cast + transpose -> xtg[d] [128, C] bf16
            if dbg is not None:
                xg0 = pp.tile([128, D], F32, tag="dxg0")
                nc.gpsimd.indirect_dma_start(
                    out=xg0[:], out_offset=None, in_=xpad[:],
                    in_offset=IndirectOffsetOnAxis(ap=idxs[0][:], axis=0))
                nc.sync.dma_start(dbg["d_xg0"][:], xg0[:])
            xtg = [pp.tile([128, C], BF16, tag=f"xtg{d}", name=f"xtg{d}")
                   for d in range(ND)]
            for k in range(NS):
                xg = sxg.tile([128, D], F32, tag="xg")
                nc.gpsimd.indirect_dma_start(
                    out=xg[:], out_offset=None, in_=xpad[:],
                    in_offset=IndirectOffsetOnAxis(ap=idxs[k][:], axis=0))
                xgb = sxb.tile([128, D], BF16, tag="xgb")
                nc.scalar.activation(xgb[:], xg[:], AF.Copy,
                                     scale=pairs[k][:, 1:2])
                for d in range(ND):
                    pt = pfs.tile([128, 128], BF16, tag="tb")
                    nc.tensor.transpose(
                        pt[:], xgb[:, d * 128:(d + 1) * 128], ident_b[:])
                    nc.vector.tensor_copy(
                        xtg[d][:, k * 128:(k + 1) * 128], pt[:])

            # fc1: hT[j] [128, C] bf16 = relu(w1.T x + b1)
            ht = [pp.tile([128, C], BF16, tag=f"ht{j}", name=f"ht{j}")
                  for j in range(NH)]
            for j in range(NH):
                pm = pfm.tile([128, C], F32, tag="mm")
                for d in range(ND):
                    mm_split(nc, pm[:], w1sb[d][:, j * 128:(j + 1) * 128],
                             xtg[d][:], start=(d == 0), stop=(d == ND - 1))
                nc.scalar.activation(ht[j][:], pm[:], AF.Relu,
                                     bias=b1sb[:, j:j + 1])

            # p as rows for the rank-1 b2 term
            prow = []
            for k in range(NS):
                pt = pfs.tile([1, 128], F32, tag="tb")
                nc.tensor.transpose(pt[:], pairs[k][:, 1:2], ident_f[:])
                pr = pp.tile([1, 128], F32, tag=f"prow{k}")
                nc.vector.tensor_copy(pr[:], pt[:])
                prow.append(pr)

            # fc2: y[t, d] accumulated in sbuf over 4 h-groups
            yac = [pp.tile([128, D], F32, tag=f"yac{k}", name=f"yac{k}")
                   for k in range(NS)]
            NG = 4
            per = NH // NG
            for g in range(NG):
                w2g = []
                for jj in range(per):
                    j = g * per + jj
                    wt = sw2.tile([128, D], BF16, tag="w2")
                    nc.sync.dma_start(wt[:], w2t[j * 128:(j + 1) * 128, :])
                    w2g.append(wt)
                for k in range(NS):
                    py = pfm.tile([128, D], F32, tag="mm")
                    if g == 0:
                        mm_split(nc, py[:], prow[k][:], b2sb[:],
                                 start=True, stop=False)
                    for jj in range(per):
                        j = g * per + jj
                        mm_split(nc, py[:], ht[j][:, k * 128:(k + 1) * 128],
                                 w2g[jj][:],
                                 start=(g != 0 and jj == 0),
                                 stop=(jj == per - 1))
                    if g == 0:
                        nc.vector.tensor_copy(yac[k][:], py[:])
                    else:
                        nc.vector.tensor_add(yac[k][:], yac[k][:], py[:])

            if dbg is not None:
                nc.sync.dma_start(dbg["d_yac0"][:], yac[0][:])

            # scatter partial rows back
            for k in range(NS):
                nc.gpsimd.indirect_dma_start(
                    out=part[:],
                    out_offset=IndirectOffsetOnAxis(ap=idxs[k][:], axis=0),
                    in_=yac[k][:], in_offset=None)


def build_nc(cfg: Cfg):
    nc = bacc.Bacc("TRN2", target_bir_lowering=False, debug=False,
                   num_devices=8)
    io = {
        "xpad": nc.dram_tensor("xpad", [cfg.NPAD, cfg.D], F32,
                               kind="ExternalInput").ap(),
        "xT": nc.dram_tensor("xT", [cfg.D, cfg.N], F32,
                             kind="ExternalInput").ap(),
        "wgT": nc.dram_tensor("wgT", [cfg.D, cfg.E], F32,
                              kind="ExternalInput").ap(),
        "w1t": nc.dram_tensor("w1t", [cfg.D, cfg.H], BF16,
                              kind="ExternalInput").ap(),
        "w2t": nc.dram_tensor("w2t", [cfg.H, cfg.D], BF16,
                              kind="ExternalInput").ap(),
        "b1c": nc.dram_tensor("b1c", [cfg.H, 1], F32,
                              kind="ExternalInput").ap(),
        "b2r": nc.dram_tensor("b2r", [1, cfg.D], F32,
                              kind="ExternalInput").ap(),
        "e0f": nc.dram_tensor("e0f", [128, 1], F32,
                              kind="ExternalInput").ap(),
        "io8": nc.dram_tensor("io8", [128, cfg.E], F32,
                              kind="ExternalInput").ap(),
        "iotok": nc.dram_tensor("iotok", [128, cfg.NCH], F32,
                                kind="ExternalInput").ap(),
        "ltri": nc.dram_tensor("ltri", [cfg.NCH, cfg.NCH], F32,
                               kind="ExternalInput").ap(),
        "part": nc.dram_tensor("part", [cfg.NPAD, cfg.D], F32,
                               kind="ExternalOutput").ap(),
        "aux": nc.dram_tensor("aux", [1, 1], F32,
                              kind="ExternalOutput").ap(),
    }
    with tile.TileContext(nc) as tc:
        build_moe(tc, cfg, io)
    nc.compile()
    return nc


def make_in_maps(cfg: Cfg, x, Wg, W1, b1, W2, b2):
    N, D, H, E = cfg.N, cfg.D, cfg.H, cfg.E
    xr = np.ascontiguousarray(np.asarray(x, np.float32).reshape(N, D))
    xpad = np.zeros((cfg.NPAD, D), np.float32)
    xpad[:N] = xr
    xT = np.ascontiguousarray(xr.T)
    wgT = np.ascontiguousarray(np.asarray(Wg, np.float32).T)
    W1 = np.asarray(W1, np.float32)
    W2 = np.asarray(W2, np.float32)
    b1 = np.asarray(b1, np.float32)
    b2 = np.asarray(b2, np.float32)
    io8 = np.tile(np.arange(E, dtype=np.float32), (128, 1))
    iotok = (np.arange(cfg.NCH, dtype=np.float32)[None, :] * 128
             + np.arange(128, dtype=np.float32)[:, None])
    iotok = np.ascontiguousarray(iotok)
    in_maps = []
    for e in range(E):
        in_maps.append({
            "xpad": xpad,
            "xT": xT,
            "wgT": wgT,
            "w1t": np.ascontiguousarray(W1[e].T).astype(ml_dtypes.bfloat16),
            "w2t": np.ascontiguousarray(W2[e].T).astype(ml_dtypes.bfloat16),
            "b1c": np.ascontiguousarray(b1[e].reshape(H, 1)),
            "b2r": np.ascontiguousarray(b2[e].reshape(1, D)),
            "e0f": np.full((128, 1), float(e), np.float32),
            "io8": io8,
            "iotok": iotok,
            "ltri": np.triu(np.ones((cfg.NCH, cfg.NCH), np.float32), 1),
        })
    return in_maps


def _install_ntff_hook_shim():
    """The agent image's antenv lacks axon_hooks; recreate it so
    run_bass_kernel_spmd(trace=True) can NTFF-profile via axon."""
    import sys, types
    if "antenv.axon_hooks" in sys.modules:
        return
    try:
        from trn_agent_boot.trn_boot import _ntff_profile_via_ctypes
        mod = types.ModuleType("antenv.axon_hooks")
        mod._hook = _ntff_profile_via_ctypes("/opt/axon/libaxon_pjrt.so")
        mod.set_axon_ntff_profile_hook = lambda h: setattr(mod, "_hook", h)
        mod.get_axon_ntff_profile_hook = lambda: mod._hook
        sys.modules["antenv.axon_hooks"] = mod
        import antenv
        antenv.axon_hooks = mod
    except Exception as e:  # profiling is best-effort
        print(f"ntff hook shim unavailable: {e}")


_NC_CACHE = {}


def _get_nc(cfg: Cfg):
    key = (cfg.N, cfg.D, cfg.H, cfg.E, cfg.C)
    if key not in _NC_CACHE:
        _NC_CACHE[key] = build_nc(cfg)
    return _NC_CACHE[key]


def kernel(x, Wg, W1, b1, W2, b2, _trace=False):
    cfg = Cfg()
    nc = _get_nc(cfg)
    if _trace:
        _install_ntff_hook_shim()
    in_maps = make_in_maps(cfg, x, Wg, W1, b1, W2, b2)
    res = bass_utils.run_bass_kernel_spmd(
        nc, in_maps, core_ids=list(range(cfg.E)), trace=_trace)
    out = np.zeros((cfg.N, cfg.D), np.float32)
    for r in res.results:
        out += r["part"][:cfg.N]
    aux = np.float32(res.results[0]["aux"][0, 0])
    out = out.reshape(2, 1024, 1024)
    if _trace:
        return (out, aux), res
    return out, aux


# revision 28
# speedup vs baseline: 1.9762x; 1.9762x over previous
"""MoE top-2 routing kernel for trn2, expert-parallel over 8 cores.

B=2, S=1024, D=1024, H=4096, E=8, K=2.  Each core hosts one expert
(pre-transposed bf16 weights), computes the f32 router replicated,
compacts its routed tokens (<=C=640) via cumsum + indirect-DMA scatter,
gathers token rows, runs fc1/relu/fc2 in bf16 (prob-scaling folded into
the gathered activations), scatters partial outputs back to token rows.
Host sums the 8 partials. aux_loss computed on device (core 0's copy).
"""

import numpy as np
import ml_dtypes

import concourse.bass as bass
import concourse.bacc as bacc
import concourse.tile as tile
import concourse.mybir as mybir
from concourse import bass_utils
from concourse.bass import IndirectOffsetOnAxis
from concourse.masks import make_identity

F32 = mybir.dt.float32
BF16 = mybir.dt.bfloat16
I32 = mybir.dt.int32
U32 = mybir.dt.uint32
AF = mybir.ActivationFunctionType
ALU = mybir.AluOpType


def mm_split(nc, out, lhsT, rhs, start, stop, nmax=512):
    """matmul with out columns split so each inst stays within a PSUM bank."""
    n = out.shape[-1]
    for s in range(0, n, nmax):
        e = min(s + nmax, n)
        nc.tensor.matmul(out[:, s:e], lhsT=lhsT, rhs=rhs[:, s:e],
                         start=start, stop=stop)


class Cfg:
    def __init__(self, N=2048, D=1024, H=4096, E=8, C=640):
        self.N, self.D, self.H, self.E, self.C = N, D, H, E, C
        self.NCH = N // 128      # token chunks
        self.ND = D // 128       # d chunks
        self.NH = H // 128       # h chunks
        self.NS = C // 128       # slot tiles
        self.NPAD = N + 128      # x rows incl. dump row at N
        self.DUMP = N            # dump row index


def build_moe(tc: tile.TileContext, cfg: Cfg, io: dict, dbg: dict | None = None):
    nc = tc.nc
    N, D, H, E, C = cfg.N, cfg.D, cfg.H, cfg.E, cfg.C
    NCH, ND, NH, NS = cfg.NCH, cfg.ND, cfg.NH, cfg.NS

    xpad, xT, wgT = io["xpad"], io["xT"], io["wgT"]
    w1t, w2t, b1c, b2r = io["w1t"], io["w2t"], io["b1c"], io["b2r"]
    e0f, io8, iotok = io["e0f"], io["io8"], io["iotok"]
    part, aux = io["part"], io["aux"]

    with tc.tile_pool(name="persist", bufs=1) as pp, \
         tc.tile_pool(name="w1pool", bufs=1) as pw1, \
         tc.tile_pool(name="dram", bufs=1, space="DRAM") as pdram:
        # ---- constants / small inputs ----
        ident_f = pp.tile([128, 128], F32, tag="identf")
        make_identity(nc, ident_f[:])
        ident_b = pp.tile([128, 128], BF16, tag="identb")
        make_identity(nc, ident_b[:])
        # strict upper-tri ones [NCH, NCH]: L[i,j]=1 iff i<j (for excl cumsum)
        lst = pp.tile([NCH, NCH], F32, tag="lst")
        nc.sync.dma_start(lst[:], io["ltri"][:])
        zrow = pp.tile([NCH, 128], F32, tag="zrow")
        nc.vector.memset(zrow[:], 0.0)
        ones = pp.tile([128, 1], F32, tag="ones")
        nc.vector.memset(ones[:], 1.0)

        b1sb = pp.tile([128, NH], F32, tag="b1sb")
        nc.sync.dma_start(b1sb[:].rearrange("p (c o) -> p c o", o=1),
                          b1c.rearrange("(c p) o -> p c o", p=128))
        b2sb = pp.tile([1, D], F32, tag="b2sb")
        nc.sync.dma_start(b2sb[:], b2r[:])
        e0sb = pp.tile([128, 1], F32, tag="e0sb")
        nc.sync.dma_start(e0sb[:], e0f[:])
        io8sb = pp.tile([128, E], F32, tag="io8sb")
        nc.sync.dma_start(io8sb[:], io8[:])
        itok = pp.tile([128, NCH], F32, tag="itok")
        nc.sync.dma_start(itok[:], iotok[:])
        wgsb = pp.tile([128, ND * E], F32, tag="wgsb")
        nc.sync.dma_start(wgsb[:].rearrange("p (c e) -> p c e", e=cfg.E),
                          wgT.rearrange("(c p) e -> p c e", p=128))

        # =================== phase R: router + tables ===================
        with tc.tile_pool(name="pr_small", bufs=2, space="PSUM") as prs, \
             tc.tile_pool(name="sb_xt", bufs=2) as sxt, \
             tc.tile_pool(name="sb_lg", bufs=1) as slg:
            # logits [E, N] = wgT.T @ xT  (f32)
            lcopy = slg.tile([E, N], F32, tag="lcopy")
            with tc.tile_pool(name="pr_logit", bufs=1, space="PSUM") as prl:
                lps = prl.tile([E, N], F32, tag="lps")
                for d in range(ND):
                    xts = sxt.tile([128, N], F32, tag="xt")
                    nc.sync.dma_start(xts[:], xT[d * 128:(d + 1) * 128, :])
                    for t4 in range(N // 512):
                        nc.tensor.matmul(
                            lps[:, t4 * 512:(t4 + 1) * 512],
                            lhsT=wgsb[:, d * E:(d + 1) * E],
                            rhs=xts[:, t4 * 512:(t4 + 1) * 512],
                            start=(d == 0), stop=(d == ND - 1))
                nc.vector.tensor_copy(lcopy[:], lps[:])
            # transpose to [t, e] per 128-token chunk
            lte = pp.tile([128, NCH * E], F32, tag="lte")
            for c in range(NCH):
                pt = prs.tile([128, E], F32, tag="tp")
                nc.tensor.transpose(pt[:], lcopy[:, c * 128:(c + 1) * 128],
                                    ident_f[0:E, 0:E])
                nc.vector.tensor_copy(lte[:, c * E:(c + 1) * E], pt[:])

            # top-2 (sorted top-8) per token
            mv = pp.tile([128, NCH * E], F32, tag="mv")
            mi = pp.tile([128, NCH * E], U32, tag="mi")
            for c in range(NCH):
                sl = slice(c * E, (c + 1) * E)
                nc.vector.max(out=mv[:, sl], in_=lte[:, sl])
                nc.vector.max_index(out=mi[:, sl], in_max=mv[:, sl],
                                    in_values=lte[:, sl])
            mv3 = mv[:].rearrange("p (c e) -> p c e", e=E)
            mi3 = mi[:].rearrange("p (c e) -> p c e", e=E)
            e1f = pp.tile([128, NCH], F32, tag="e1f")
            e2f = pp.tile([128, NCH], F32, tag="e2f")
            nc.vector.tensor_copy(e1f[:], mi3[:, :, 0])
            nc.vector.tensor_copy(e2f[:], mi3[:, :, 1])
            d21 = pp.tile([128, NCH], F32, tag="d21")
            nc.vector.tensor_tensor(out=d21[:], in0=mv3[:, :, 1],
                                    in1=mv3[:, :, 0], op=ALU.subtract)
            p2 = pp.tile([128, NCH], F32, tag="p2")
            nc.scalar.activation(p2[:], d21[:], AF.Sigmoid)
            p1 = pp.tile([128, NCH], F32, tag="p1")
            nc.scalar.activation(p1[:], p2[:], AF.Copy, bias=1.0, scale=-1.0)

            # my-expert mask + prob
            my1 = pp.tile([128, NCH], F32, tag="my1")
            my2 = pp.tile([128, NCH], F32, tag="my2")
            nc.vector.tensor_scalar(out=my1[:], in0=e1f[:], scalar1=e0sb[:, 0:1],
                                    scalar2=None, op0=ALU.is_equal)
            nc.vector.tensor_scalar(out=my2[:], in0=e2f[:], scalar1=e0sb[:, 0:1],
                                    scalar2=None, op0=ALU.is_equal)
            mym = pp.tile([128, NCH], F32, tag="mym")
            nc.vector.tensor_add(mym[:], my1[:], my2[:])
            t1 = pp.tile([128, NCH], F32, tag="t1")
            t2 = pp.tile([128, NCH], F32, tag="t2")
            nc.vector.tensor_mul(t1[:], my1[:], p1[:])
            nc.vector.tensor_mul(t2[:], my2[:], p2[:])
            myrp = pp.tile([128, NCH], F32, tag="myrp")
            nc.vector.tensor_add(myrp[:], t1[:], t2[:])

            # ---- aux loss ----
            sh = pp.tile([128, NCH * E], F32, tag="sh")
            for c in range(NCH):
                sl = slice(c * E, (c + 1) * E)
                nc.vector.tensor_scalar(out=sh[:, sl], in0=lte[:, sl],
                                        scalar1=mv[:, c * E:c * E + 1],
                                        scalar2=None, op0=ALU.subtract)
            ep = pp.tile([128, NCH * E], F32, tag="ep")
            nc.scalar.activation(ep[:], sh[:], AF.Exp)
            zs = pp.tile([128, NCH], F32, tag="zs")
            nc.vector.reduce_sum(
                out=zs[:].rearrange("p (c o) -> p c o", o=1),
                in_=ep[:].rearrange("p (c e) -> p c e", e=E),
                axis=mybir.AxisListType.X)
            zi = pp.tile([128, NCH], F32, tag="zi")
            nc.vector.reciprocal(zi[:], zs[:])
            prb = pp.tile([128, NCH * E], F32, tag="prb")
            for c in range(NCH):
                sl = slice(c * E, (c + 1) * E)
                nc.vector.tensor_scalar(out=prb[:, sl], in0=ep[:, sl],
                                        scalar1=zi[:, c:c + 1],
                                        scalar2=None, op0=ALU.mult)
            pracc = pp.tile([128, E], F32, tag="pracc")
            nc.vector.reduce_sum(
                out=pracc[:].rearrange("p (e o) -> p e o", o=1),
                in_=prb[:].rearrange("p (c e) -> p e c", e=E),
                axis=mybir.AxisListType.X)
            eq1 = pp.tile([128, NCH * E], F32, tag="eq1")
            eq2 = pp.tile([128, NCH * E], F32, tag="eq2")
            for c in range(NCH):
                sl = slice(c * E, (c + 1) * E)
                nc.vector.tensor_scalar(out=eq1[:, sl], in0=io8sb[:],
                                        scalar1=e1f[:, c:c + 1],
                                        scalar2=None, op0=ALU.is_equal)
                nc.vector.tensor_scalar(out=eq2[:, sl], in0=io8sb[:],
                                        scalar1=e2f[:, c:c + 1],
                                        scalar2=None, op0=ALU.is_equal)
            eqs = pp.tile([128, NCH * E], F32, tag="eqs")
            nc.vector.tensor_add(eqs[:], eq1[:], eq2[:])
            cnt8 = pp.tile([128, E], F32, tag="cnt8")
            nc.vector.reduce_sum(
                out=cnt8[:].rearrange("p (e o) -> p e o", o=1),
                in_=eqs[:].rearrange("p (c e) -> p e c", e=E),
                axis=mybir.AxisListType.X)
            psa = prs.tile([1, 2 * E], F32, tag="tp")
            nc.tensor.matmul(psa[:, 0:E], lhsT=ones[:], rhs=pracc[:],
                             start=True, stop=True)
            nc.tensor.matmul(psa[:, E:2 * E], lhsT=ones[:], rhs=cnt8[:],
                             start=True, stop=True)
            acop = pp.tile([1, 2 * E], F32, tag="acop")
            nc.vector.tensor_copy(acop[:], psa[:])
            am = pp.tile([1, E], F32, tag="am")
            nc.vector.tensor_mul(am[:], acop[:, 0:E], acop[:, E:2 * E])
            asm = pp.tile([1, 1], F32, tag="asm")
            nc.vector.reduce_sum(out=asm[:], in_=am[:],
                                 axis=mybir.AxisListType.X)
            auxsb = pp.tile([1, 1], F32, tag="auxsb")
            nc.scalar.activation(auxsb[:], asm[:], AF.Copy,
                                 scale=float(E) / float(N) / float(N))
            nc.sync.dma_start(aux[:], auxsb[:])

            # ---- compaction: global positions of my tokens ----
            pmt = prs.tile([NCH, 128], F32, tag="tpw")
            nc.tensor.transpose(pmt[:], mym[:], ident_f[:])
            mmt = pp.tile([NCH, 128], F32, tag="mmt")
            nc.vector.tensor_copy(mmt[:], pmt[:])
            posw = pp.tile([NCH, 128], F32, tag="posw")
            nc.vector.tensor_tensor_scan(
                out=posw[:], data0=mmt[:], data1=zrow[:], initial=0.0,
                op0=ALU.add, op1=ALU.max)
            tot = pp.tile([NCH, 1], F32, tag="tot")
            nc.vector.tensor_copy(tot[:], posw[:, 127:128])
            pb = prs.tile([NCH, 1], F32, tag="tpw")
            nc.tensor.matmul(pb[:], lhsT=lst[:], rhs=tot[:], start=True,
                             stop=True)
            base = pp.tile([NCH, 1], F32, tag="base")
            nc.vector.tensor_copy(base[:], pb[:])
            posg = pp.tile([NCH, 128], F32, tag="posg")
            nc.vector.tensor_scalar(out=posg[:], in0=posw[:],
                                    scalar1=base[:, 0:1], scalar2=None,
                                    op0=ALU.add)
            pposf = prs.tile([128, NCH], F32, tag="tp")
            nc.tensor.transpose(pposf[:], posg[:], ident_f[0:NCH, 0:NCH])
            posf = pp.tile([128, NCH], F32, tag="posf")
            nc.vector.tensor_copy(posf[:], pposf[:])

            # offsets: mym ? pos-1 : DUMP
            o1 = pp.tile([128, NCH], F32, tag="o1")
            nc.vector.tensor_scalar(out=o1[:], in0=posf[:],
                                    scalar1=-(1.0 + cfg.DUMP), scalar2=None,
                                    op0=ALU.add)
            o2 = pp.tile([128, NCH], F32, tag="o2")
            nc.vector.tensor_mul(o2[:], o1[:], mym[:])
            off = pp.tile([128, NCH], F32, tag="off")
            nc.vector.tensor_scalar(out=off[:], in0=o2[:],
                                    scalar1=float(cfg.DUMP), scalar2=None,
                                    op0=ALU.add)

            # (tokenid, prob, occupancy) triples per token
            vals = pp.tile([128, NCH * 3], F32, tag="vals")
            v3 = vals[:].rearrange("p (c v) -> p c v", v=3)
            nc.vector.tensor_copy(v3[:, :, 0], itok[:])
            nc.vector.tensor_copy(v3[:, :, 1], myrp[:])
            nc.vector.memset(v3[:, :, 2], 1.0)

            # compact to slot tables via PE selection matmuls:
            # pair_k[s, :] = sum_t (off[t] == k*128+s) * vals[t, :]
            io128 = pp.tile([128, 128], F32, tag="io128")
            nc.sync.dma_start(io128[:], io["io128"][:])
            pairs, idxs = [], []
            with tc.tile_pool(name="sb_eq", bufs=3) as seq, \
                 tc.tile_pool(name="pr_pair", bufs=2, space="PSUM") as prp:
                for k in range(NS):
                    offk = pp.tile([128, NCH], F32, tag="offk",
                                   name=f"offk{k}")
                    nc.vector.tensor_scalar(
                        out=offk[:], in0=off[:], scalar1=float(-k * 128),
                        scalar2=None, op0=ALU.add)
                    ppair = prp.tile([128, 3], F32, tag="tpair")
                    for c in range(NCH):
                        eq = seq.tile([128, 128], F32, tag="eq")
                        nc.vector.tensor_scalar(
                            out=eq[:], in0=io128[:],
                            scalar1=offk[:, c:c + 1], scalar2=None,
                            op0=ALU.is_equal)
                        nc.tensor.matmul(
                            ppair[:], lhsT=eq[:],
                            rhs=v3[:, c, :], start=(c == 0),
                            stop=(c == NCH - 1))
                    pk = pp.tile([128, 3], F32, tag=f"pair{k}",
                                 name=f"pair{k}")
                    nc.vector.tensor_copy(pk[:], ppair[:])
                    # empty slots -> dump row: idx = tok + (1-occ)*DUMP
                    adj = pp.tile([128, 1], F32, tag="adj", name=f"adj{k}")
                    nc.scalar.activation(adj[:], pk[:, 2:3], AF.Copy,
                                         scale=-float(cfg.DUMP),
                                         bias=float(cfg.DUMP))
                    idxf = pp.tile([128, 1], F32, tag="idxf",
                                   name=f"idxf{k}")
                    nc.vector.tensor_add(idxf[:], pk[:, 0:1], adj[:])
                    ik = pp.tile([128, 1], I32, tag=f"idx{k}",
                                 name=f"idx{k}")
                    nc.vector.tensor_copy(ik[:], idxf[:])
                    pairs.append(pk)
                    idxs.append(ik)

            if dbg is not None:
                nc.sync.dma_start(dbg["d_p1"][:], p1[:])
                nc.sync.dma_start(dbg["d_p2"][:], p2[:])
                nc.sync.dma_start(dbg["d_myrp"][:], myrp[:])
                nc.sync.dma_start(dbg["d_mym"][:], mym[:])
                nc.sync.dma_start(dbg["d_posf"][:], posf[:])
                nc.sync.dma_start(dbg["d_offi"][:], off[:])
                nc.sync.dma_start(dbg["d_vals"][:], vals[:])
                for k in range(NS):
                    nc.sync.dma_start(
                        dbg["d_tab"][k * 128:(k + 1) * 128, :], pairs[k][:])

        # w1 resident: ND tiles [128, H] bf16 (loaded after router xT DMAs)
        w1sb = []
        for d in range(ND):
            t = pw1.tile([128, H], BF16, tag=f"w1_{d}", name=f"w1_{d}")
            nc.sync.dma_start(t[:], w1t[d * 128:(d + 1) * 128, :])
            w1sb.append(t)

        # =================== phase F: gather, fc1, fc2, scatter =========
        with tc.tile_pool(name="pf_mm", bufs=3, space="PSUM") as pfm, \
             tc.tile_pool(name="pf_small", bufs=2, space="PSUM") as pfs, \
             tc.tile_pool(name="sb_xg", bufs=2) as sxg, \
             tc.tile_pool(name="sb_xgb", bufs=2) as sxb, \
             tc.tile_pool(name="sb_w2", bufs=10) as sw2:
            # gather + prob-scale + cast + transpose -> xtg[d] [128, C] bf16
            if dbg is not None:
                xg0 = pp.tile([128, D], F32, tag="dxg0")
                nc.gpsimd.indirect_dma_start(
                    out=xg0[:], out_offset=None, in_=xpad[:],
                    in_offset=IndirectOffsetOnAxis(ap=idxs[0][:], axis=0))
                nc.sync.dma_start(dbg["d_xg0"][:], xg0[:])
            xtg = [pp.tile([128, C], BF16, tag=f"xtg{d}", name=f"xtg{d}")
                   for d in range(ND)]
            for k in range(NS):
                xg = sxg.tile([128, D], F32, tag="xg")
                nc.gpsimd.indirect_dma_start(
                    out=xg[:], out_offset=None, in_=xpad[:],
                    in_offset=IndirectOffsetOnAxis(ap=idxs[k][:], axis=0))
                xgb = sxb.tile([128, D], BF16, tag="xgb")
                nc.scalar.activation(xgb[:], xg[:], AF.Copy,
                                     scale=pairs[k][:, 1:2])
                for d in range(ND):
                    pt = pfs.tile([128, 128], BF16, tag="tb")
                    nc.tensor.transpose(
                        pt[:], xgb[:, d * 128:(d + 1) * 128], ident_b[:])
                    nc.vector.tensor_copy(
                        xtg[d][:, k * 128:(k + 1) * 128], pt[:])

            # fc1: hT[j] [128, C] bf16 = relu(w1.T x + b1)
            ht = [pp.tile([128, C], BF16, tag=f"ht{j}", name=f"ht{j}")
                  for j in range(NH)]
            for j in range(NH):
                pm = pfm.tile([128, C], F32, tag="mm")
                for d in range(ND):
                    mm_split(nc, pm[:], w1sb[d][:, j * 128:(j + 1) * 128],
                             xtg[d][:], start=(d == 0), stop=(d == ND - 1))
                nc.scalar.activation(ht[j][:], pm[:], AF.Relu,
                                     bias=b1sb[:, j:j + 1])

            # p as rows for the rank-1 b2 term
            prow = []
            for k in range(NS):
                pt = pfs.tile([1, 128], F32, tag="tb")
                nc.tensor.transpose(pt[:], pairs[k][:, 1:2], ident_f[:])
                pr = pp.tile([1, 128], F32, tag=f"prow{k}")
                nc.vector.tensor_copy(pr[:], pt[:])
                prow.append(pr)

            # fc2: y[t, d] accumulated in sbuf over 4 h-groups
            yac = [pp.tile([128, D], F32, tag=f"yac{k}", name=f"yac{k}")
                   for k in range(NS)]
            NG = 4
            per = NH // NG
            for g in range(NG):
                w2g = []
                for jj in range(per):
                    j = g * per + jj
                    wt = sw2.tile([128, D], BF16, tag="w2")
                    nc.sync.dma_start(wt[:], w2t[j * 128:(j + 1) * 128, :])
                    w2g.append(wt)
                for k in range(NS):
                    py = pfm.tile([128, D], F32, tag="mm")
                    if g == 0:
                        mm_split(nc, py[:], prow[k][:], b2sb[:],
                                 start=True, stop=False)
                    for jj in range(per):
                        j = g * per + jj
                        mm_split(nc, py[:], ht[j][:, k * 128:(k + 1) * 128],
                                 w2g[jj][:],
                                 start=(g != 0 and jj == 0),
                                 stop=(jj == per - 1))
                    if g == 0:
                        nc.vector.tensor_copy(yac[k][:], py[:])
                    else:
                        nc.vector.tensor_add(yac[k][:], yac[k][:], py[:])

            if dbg is not None:
                nc.sync.dma_start(dbg["d_yac0"][:], yac[0][:])

            # scatter partial rows back
            for k in range(NS):
                nc.gpsimd.indirect_dma_start(
                    out=part[:],
                    out_offset=IndirectOffsetOnAxis(ap=idxs[k][:], axis=0),
                    in_=yac[k][:], in_offset=None)


def build_nc(cfg: Cfg):
    nc = bacc.Bacc("TRN2", target_bir_lowering=False, debug=False,
                   num_devices=8)
    io = {
        "xpad": nc.dram_tensor("xpad", [cfg.NPAD, cfg.D], F32,
                               kind="ExternalInput").ap(),
        "xT": nc.dram_tensor("xT", [cfg.D, cfg.N], F32,
                             kind="ExternalInput").ap(),
        "wgT": nc.dram_tensor("wgT", [cfg.D, cfg.E], F32,
                              kind="ExternalInput").ap(),
        "w1t": nc.dram_tensor("w1t", [cfg.D, cfg.H], BF16,
                              kind="ExternalInput").ap(),
        "w2t": nc.dram_tensor("w2t", [cfg.H, cfg.D], BF16,
                              kind="ExternalInput").ap(),
        "b1c": nc.dram_tensor("b1c", [cfg.H, 1], F32,
                              kind="ExternalInput").ap(),
        "b2r": nc.dram_tensor("b2r", [1, cfg.D], F32,
                              kind="ExternalInput").ap(),
        "e0f": nc.dram_tensor("e0f", [128, 1], F32,
                              kind="ExternalInput").ap(),
        "io8": nc.dram_tensor("io8", [128, cfg.E], F32,
                              kind="ExternalInput").ap(),
        "iotok": nc.dram_tensor("iotok", [128, cfg.NCH], F32,
                                kind="ExternalInput").ap(),
        "ltri": nc.dram_tensor("ltri", [cfg.NCH, cfg.NCH], F32,
                               kind="ExternalInput").ap(),
        "io128": nc.dram_tensor("io128", [128, 128], F32,
                                kind="ExternalInput").ap(),
        "part": nc.dram_tensor("part", [cfg.NPAD, cfg.D], F32,
                               kind="ExternalOutput").ap(),
        "aux": nc.dram_tensor("aux", [1, 1], F32,
                              kind="ExternalOutput").ap(),
    }
    with tile.TileContext(nc) as tc:
        build_moe(tc, cfg, io)
    nc.compile()
    return nc


def make_in_maps(cfg: Cfg, x, Wg, W1, b1, W2, b2):
    N, D, H, E = cfg.N, cfg.D, cfg.H, cfg.E
    xr = np.ascontiguousarray(np.asarray(x, np.float32).reshape(N, D))
    xpad = np.zeros((cfg.NPAD, D), np.float32)
    xpad[:N] = xr
    xT = np.ascontiguousarray(xr.T)
    wgT = np.ascontiguousarray(np.asarray(Wg, np.float32).T)
    W1 = np.asarray(W1, np.float32)
    W2 = np.asarray(W2, np.float32)
    b1 = np.asarray(b1, np.float32)
    b2 = np.asarray(b2, np.float32)
    io8 = np.tile(np.arange(E, dtype=np.float32), (128, 1))
    iotok = (np.arange(cfg.NCH, dtype=np.float32)[None, :] * 128
             + np.arange(128, dtype=np.float32)[:, None])
    iotok = np.ascontiguousarray(iotok)
    in_maps = []
    for e in range(E):
        in_maps.append({
            "xpad": xpad,
            "xT": xT,
            "wgT": wgT,
            "w1t": np.ascontiguousarray(W1[e].T).astype(ml_dtypes.bfloat16),
            "w2t": np.ascontiguousarray(W2[e].T).astype(ml_dtypes.bfloat16),
            "b1c": np.ascontiguousarray(b1[e].reshape(H, 1)),
            "b2r": np.ascontiguousarray(b2[e].reshape(1, D)),
            "e0f": np.full((128, 1), float(e), np.float32),
            "io8": io8,
            "iotok": iotok,
            "ltri": np.triu(np.ones((cfg.NCH, cfg.NCH), np.float32), 1),
            "io128": np.tile(np.arange(128, dtype=np.float32), (128, 1)),
        })
    return in_maps


def _install_ntff_hook_shim():
    """The agent image's antenv lacks axon_hooks; recreate it so
    run_bass_kernel_spmd(trace=True) can NTFF-profile via axon."""
    import sys, types
    if "antenv.axon_hooks" in sys.modules:
        return
    try:
        from trn_agent_boot.trn_boot import _ntff_profile_via_ctypes
        mod = types.ModuleType("antenv.axon_hooks")
        mod._hook = _ntff_profile_via_ctypes("/opt/axon/libaxon_pjrt.so")
        mod.set_axon_ntff_profile_hook = lambda h: setattr(mod, "_hook", h)
        mod.get_axon_ntff_profile_hook = lambda: mod._hook
        sys.modules["antenv.axon_hooks"] = mod
        import antenv
        antenv.axon_hooks = mod
    except Exception as e:  # profiling is best-effort
        print(f"ntff hook shim unavailable: {e}")


_NC_CACHE = {}


def _get_nc(cfg: Cfg):
    key = (cfg.N, cfg.D, cfg.H, cfg.E, cfg.C)
    if key not in _NC_CACHE:
        _NC_CACHE[key] = build_nc(cfg)
    return _NC_CACHE[key]


def kernel(x, Wg, W1, b1, W2, b2, _trace=False):
    cfg = Cfg()
    nc = _get_nc(cfg)
    if _trace:
        _install_ntff_hook_shim()
    in_maps = make_in_maps(cfg, x, Wg, W1, b1, W2, b2)
    res = bass_utils.run_bass_kernel_spmd(
        nc, in_maps, core_ids=list(range(cfg.E)), trace=_trace)
    out = np.zeros((cfg.N, cfg.D), np.float32)
    for r in res.results:
        out += r["part"][:cfg.N]
    aux = np.float32(res.results[0]["aux"][0, 0])
    out = out.reshape(2, 1024, 1024)
    if _trace:
        return (out, aux), res
    return out, aux
